# revision 1
# baseline (speedup 1.0000x reference)
"""Trainium2 Bass kernel for nn_CrossAtten: cross-attention
out = softmax((q Wq^T)(kv Wk^T)^T / sqrt(D)) @ (kv Wv^T) @ Wout^T + bout

Shapes (hardcoded): q,kv [4,16,2048,128] fp32; Wq,Wout [128,128]; Wkv [256,128]; bout [128].
Sharding: batch*heads (64 pairs) split 8 per NeuronCore across 8 cores (pure data parallel).

Algebraic restructure (host-side weight folding):
  A    = Wq^T @ Wk          -> scores S = q A kv^T     (one projected tensor u = qA)
  WvoT = Wv^T @ Wout^T      -> PV matmul directly yields final projection (pre-bias)
Softmax is computed max-free (logits ~ N(0,1), |logit| < ~7, exp is safe in fp32),
with scores produced transposed [j, i] so the PV contraction (over j) needs no
transpose of the attention matrix; denominators come from a ones-vector matmul.
PE matmuls run in float32r (TF32-class: ~1.6e-4 rel err, full 1 cycle/row speed).
"""
import sys

if "/opt/trn_rl_repo" not in sys.path:
    sys.path.insert(0, "/opt/trn_rl_repo")

from contextlib import ExitStack

import numpy as np

import concourse.bacc as bacc
import concourse.tile as tile
import concourse.mybir as mybir
from concourse.bass_utils import run_bass_kernel_spmd

B, H, I, J, D = 4, 16, 2048, 2048, 128
BH = B * H
N_CORES = 8
PER_CORE = BH // N_CORES          # 8 (b,h) pairs per core
P = 128                           # partitions
IT = I // P                       # 16 i-tiles
JT = J // P                       # 16 j-tiles
IC = 512                          # i-chunk (columns per scores/PV matmul)
NIC = I // IC                     # 4 i-chunks
SCALE = D ** -0.5

F32 = mybir.dt.float32
F32R = mybir.dt.float32r
EXP = mybir.ActivationFunctionType.Exp

_cache = {}

# tunables (cost-model sweep)
CFG = dict(ps_bufs=8, ep_bufs=4, skew=3, fdelay=2, cast_load=True)


def _build(repeat=1):
    nc = bacc.Bacc(
        "TRN2",
        target_bir_lowering=False,
        debug=False,
        enable_asserts=False,
        num_devices=N_CORES,
    )

    q_d = nc.dram_tensor("q", [PER_CORE, I, D], F32, kind="ExternalInput").ap()
    kv_d = nc.dram_tensor("kv", [PER_CORE, J, D], F32, kind="ExternalInput").ap()
    a_d = nc.dram_tensor("A", [D, D], F32, kind="ExternalInput").ap()
    wvo_d = nc.dram_tensor("WvoT", [D, 2 * D], F32, kind="ExternalInput").ap()
    boutb_d = nc.dram_tensor("bout_b", [P, D], F32, kind="ExternalInput").ap()
    ident_d = nc.dram_tensor("ident", [P, P], F32, kind="ExternalInput").ap()
    out_d = nc.dram_tensor("out", [PER_CORE, I, D], F32, kind="ExternalOutput").ap()

    with tile.TileContext(nc) as tc, ExitStack() as ctx:
        const = ctx.enter_context(tc.tile_pool(name="const", bufs=1))
        qkv = ctx.enter_context(tc.tile_pool(name="qkv", bufs=2))
        tp = ctx.enter_context(tc.tile_pool(name="tp", bufs=2))
        ep = ctx.enter_context(tc.tile_pool(name="ep", bufs=CFG["ep_bufs"]))
        fin = ctx.enter_context(tc.tile_pool(name="fin", bufs=2))
        ps = ctx.enter_context(tc.tile_pool(name="ps", bufs=CFG["ps_bufs"], space="PSUM"))

        # ---- constants (loaded / prepared once) ----
        ident = const.tile([P, P], F32, tag="ident")
        nc.sync.dma_start(ident[:], ident_d)
        bout_b = const.tile([P, D], F32, tag="bout_b")
        nc.sync.dma_start(bout_b[:], boutb_d)

        # casting SWDGE DMAs round fp32 -> f32r during the transfer
        a_r = const.tile([D, D], F32R, tag="a_r")
        nc.gpsimd.dma_start(a_r[:], a_d)
        # WvoT padded to 256 cols (zeros right half): f32r matmul needs
        # moving ap_size >= 256 for 1 cycle/row (4x otherwise)
        wvo_r = const.tile([D, 2 * D], F32R, tag="wvo_r")
        nc.gpsimd.dma_start(wvo_r[:], wvo_d)
        ones_f32 = const.tile([P, 1], F32, tag="ones_f32")
        nc.vector.memset(ones_f32[:], 1.0)
        ones_r = const.tile([P, 1], F32R, tag="ones_r")
        nc.vector.tensor_copy(ones_r[:], ones_f32[:])
        one1 = const.tile([1, 1], F32, tag="one1")
        nc.vector.memset(one1[:], 1.0)
        ident_r = const.tile([P, P], F32R, tag="ident_r")
        nc.vector.tensor_copy(ident_r[:], ident[:])

        # ============================================================
        # Emission: per-(rep,bh) "task"; task k+1's loads + setup
        # (transposes, vproj, uT) are interleaved into task k's main
        # attention pipeline so the DVE-bound setup never starves PE.
        # ============================================================
        tasks = [(r, b) for r in range(repeat) for b in range(PER_CORE)]
        SK = CFG["skew"]
        FD = CFG["fdelay"]
        TILES = {}

        def _loads(k):
            bh = tasks[k][1]
            # partition p holds rows 16p..16p+15 (i = 16p + r): 8KB contiguous
            # DRAM per partition -> ~1 descriptor per partition. Downstream,
            # transpose-tile t covers the i-set {16p + t}; the final output
            # transposes invert this for free (see store). Loads are split in
            # 4-r chunks (one per transpose group) so transposes start early;
            # kv first because setup consumes kvT first.
            ldt = F32R if CFG["cast_load"] else F32
            eng = nc.gpsimd if CFG["cast_load"] else nc.sync
            kv_sb = qkv.tile([P, J], ldt, tag="kv_sb", name=f"kv_sb_{k}")
            kvv = kv_d[bh].rearrange("(p r) d -> p r d", r=JT)
            kvs = kv_sb[:].rearrange("p (r d) -> p r d", r=JT)
            for g4 in range(0, JT, 4):
                eng.dma_start(kvs[:, g4 : g4 + 4], kvv[:, g4 : g4 + 4])
            q_sb = qkv.tile([P, I], ldt, tag="q_sb", name=f"q_sb_{k}")
            qv = q_d[bh].rearrange("(p r) d -> p r d", r=IT)
            qs = q_sb[:].rearrange("p (r d) -> p r d", r=IT)
            for g4 in range(0, IT, 4):
                eng.dma_start(qs[:, g4 : g4 + 4], qv[:, g4 : g4 + 4])
            return q_sb, kv_sb

        def _setup_steps(k, q_sb, kv_sb):
            """Closures: kvT/qT transposes (4 per PSUM slot), vproj, uT."""
            T = TILES[k] = {}
            T["qT"] = tp.tile([P, I], F32R, tag="qT", name=f"qT_{k}")
            T["kvT"] = tp.tile([P, J], F32R, tag="kvT", name=f"kvT_{k}")
            T["uT"] = tp.tile([P, I], F32R, tag="uT", name=f"uT_{k}")
            T["vproj"] = tp.tile([P, J], F32R, tag="vproj", name=f"vp_{k}")
            steps = []

            def tr_group(dst, src, g4):
                cast = CFG["cast_load"]
                pt = ps.tile([P, IC], F32R if cast else F32, tag="ps",
                             name=f"pt_{k}_{g4}")
                for t in range(4):
                    nc.tensor.transpose(
                        pt[:, t * P : (t + 1) * P],
                        src[:, (g4 + t) * P : (g4 + t + 1) * P],
                        ident_r[:] if cast else ident[:],
                    )
                nc.vector.tensor_copy(dst[:, g4 * P : (g4 + 4) * P], pt[:])

            for dst, src, nt in ((T["kvT"], kv_sb, JT), (T["qT"], q_sb, IT)):
                for g4 in range(0, nt, 4):
                    steps.append(lambda dst=dst, src=src, g4=g4: tr_group(dst, src, g4))

            def vproj_step(jt):
                pv1 = ps.tile([P, IC], F32, tag="ps", name=f"pv1_{k}_{jt}")
                nc.tensor.matmul(
                    pv1[:, 0 : 2 * P],
                    T["kvT"][:, jt * P : (jt + 1) * P],
                    wvo_r[:],
                    start=True, stop=True,
                )
                nc.vector.tensor_copy(
                    T["vproj"][:, jt * P : (jt + 1) * P], pv1[:, 0:P]
                )

            for jt in range(JT):
                steps.append(lambda jt=jt: vproj_step(jt))

            def ut_step(c):
                pu = ps.tile([P, IC], F32, tag="ps", name=f"pu_{k}_{c}")
                nc.tensor.matmul(
                    pu[:], a_r[:], T["qT"][:, c * IC : (c + 1) * IC],
                    start=True, stop=True,
                )
                nc.vector.tensor_copy(T["uT"][:, c * IC : (c + 1) * IC], pu[:])

            for c in range(NIC):
                steps.append(lambda c=c: ut_step(c))
            return steps

        def _main_pipeline(k, interleave):
            bh = tasks[k][1]
            T = TILES[k]
            kvT, uT, vproj = T["kvT"], T["uT"], T["vproj"]
            out_sb = fin.tile([P, I], F32, tag="out_sb", name=f"out_sb_{k}")
            items = [(c, jt) for c in range(NIC) for jt in range(JT)]
            s_tiles, e_tiles, pv_t, dn_t = {}, {}, {}, {}

            def _scores(c, jt):
                p_s = ps.tile([P, IC], F32, tag="ps", name=f"s_{k}_{c}_{jt}")
                nc.tensor.matmul(
                    p_s[:], kvT[:, jt * P : (jt + 1) * P],
                    uT[:, c * IC : (c + 1) * IC],
                    start=True, stop=True,
                )
                s_tiles[(c, jt)] = p_s

            def _exp(c, jt):
                e_sb = ep.tile([P, IC], F32R, tag="e_sb", name=f"e_{k}_{c}_{jt}")
                nc.scalar.activation(
                    e_sb[:], s_tiles.pop((c, jt))[:], EXP, scale=SCALE
                )
                e_tiles[(c, jt)] = e_sb

            def _pv_dn(c, jt):
                if jt == 0:
                    pv_t[c] = ps.tile([P, IC], F32, tag="ps", name=f"pv_{k}_{c}")
                    dn_t[c] = ps.tile([1, IC], F32, tag="ps", name=f"dn_{k}_{c}")
                e_sb = e_tiles.pop((c, jt))
                nc.tensor.matmul(
                    pv_t[c][:], vproj[:, jt * P : (jt + 1) * P], e_sb[:],
                    start=(jt == 0), stop=(jt == JT - 1),
                )
                nc.tensor.matmul(
                    dn_t[c][:], ones_r[:], e_sb[:],
                    start=(jt == 0), stop=(jt == JT - 1),
                )

            def _finalize(c):
                p_pv = pv_t.pop(c)
                p_dn = dn_t.pop(c)
                pvT = fin.tile([P, IC], F32R, tag="pvT", name=f"pvT_{k}_{c}")
                nc.vector.tensor_copy(pvT[:], p_pv[:])
                dn_sb = fin.tile([1, IC], F32, tag="dn_sb", name=f"dnsb_{k}_{c}")
                nc.vector.tensor_copy(dn_sb[:], p_dn[:])

                p_os = []
                for t in range(IC // P):
                    p_o = ps.tile([P, IC], F32R, tag="ps", name=f"po_{k}_{c}_{t}")
                    nc.tensor.transpose(
                        p_o[:, 0:P], pvT[:, t * P : (t + 1) * P], ident_r[:]
                    )
                    p_os.append(p_o)

                p_dt = ps.tile([P, IC], F32, tag="ps", name=f"pdt_{k}_{c}")
                for t in range(IC // P):
                    nc.tensor.matmul(
                        p_dt[:, t : t + 1],
                        dn_sb[:, t * P : (t + 1) * P],
                        one1[:],
                        start=True, stop=True,
                    )
                recip = fin.tile([P, IC // P], F32, tag="recip", name=f"rc_{k}_{c}")
                nc.vector.reciprocal(recip[:], p_dt[:, 0 : IC // P])

                for t in range(IC // P):
                    tg = c * (IC // P) + t       # global tile: i = 16p + tg
                    nc.vector.tensor_scalar_mul(
                        out_sb[:, tg * P : (tg + 1) * P],
                        p_os[t][:, 0:P],
                        recip[:, t : t + 1],
                    )
                    nc.vector.tensor_add(
                        out_sb[:, tg * P : (tg + 1) * P],
                        out_sb[:, tg * P : (tg + 1) * P],
                        bout_b[:],
                    )

            for g in range(len(items) + SK):
                if g < len(items):
                    _scores(*items[g])
                if 0 <= g - SK + 1 < len(items):
                    _exp(*items[g - SK + 1])
                if g >= SK:
                    c2, jt2 = items[g - SK]
                    _pv_dn(c2, jt2)
                    # finalize chunk c2-1 a few steps late so PE has queued
                    # scores to chew while DVE evacuates PV and denom
                    if jt2 == FD and c2 > 0:
                        _finalize(c2 - 1)
                if g % 2 == 1 and interleave:
                    interleave.pop(0)()
            _finalize(NIC - 1)
            for s in interleave:
                s()
            # single 1MB store: partition p holds rows 16p..16p+15
            nc.sync.dma_start(
                out_d[bh].rearrange("(p r) e -> p r e", r=IT),
                out_sb[:].rearrange("p (r e) -> p r e", r=IT),
            )
            del TILES[k]

        # prologue: task 0 loads + full setup
        q0 = _loads(0)
        for s in _setup_steps(0, *q0):
            s()
        for k in range(len(tasks)):
            pending = []
            if k + 1 < len(tasks):
                qn = _loads(k + 1)
                pending = _setup_steps(k + 1, *qn)
            _main_pipeline(k, pending)

    nc.compile()
    return nc


def kernel(q, kv, Wq, Wkv, Wout, bout):
    if "nc" not in _cache:
        _cache["nc"] = _build()
    nc = _cache["nc"]

    Wk = Wkv[:D].astype(np.float64)
    Wv = Wkv[D:].astype(np.float64)
    A = (Wq.astype(np.float64).T @ Wk).astype(np.float32)
    WvoT = (Wv.T @ Wout.astype(np.float64).T).astype(np.float32)
    WvoT = np.concatenate([WvoT, np.zeros((D, D), np.float32)], axis=1)
    bout_b = np.broadcast_to(np.asarray(bout, np.float32), (P, D)).copy()
    ident = np.eye(P, dtype=np.float32)

    qf = np.ascontiguousarray(np.asarray(q, np.float32).reshape(BH, I, D))
    kvf = np.ascontiguousarray(np.asarray(kv, np.float32).reshape(BH, J, D))

    in_maps = []
    for c in range(N_CORES):
        sl = slice(c * PER_CORE, (c + 1) * PER_CORE)
        in_maps.append(
            {
                "q": np.ascontiguousarray(qf[sl]),
                "kv": np.ascontiguousarray(kvf[sl]),
                "A": A,
                "WvoT": WvoT,
                "bout_b": bout_b,
                "ident": ident,
            }
        )

    global _last_in_maps
    _last_in_maps = in_maps

    res = run_bass_kernel_spmd(nc, in_maps, core_ids=list(range(N_CORES)))
    out = np.concatenate([r["out"] for r in res.results], axis=0)
    return out.reshape(B, H, I, D)


_last_in_maps = None

